# revision 14
# baseline (speedup 1.0000x reference)
"""BP-GNN message passing kernel for 8 Trainium2 NeuronCores.

Takes FULL inputs (as in reference.setup_inputs()), shards internally across
8 cores (edge parallelism by dst-range), runs 5 BP steps on device via Bass,
returns the FULL [N, C] log-belief output.

Math note: logH = -3*(1-I) collapses the per-edge logsumexp:
  raw[e,c] = log((1-a)*exp(t[e,c]) + a*S(e)),  a = e^-3, S = sum_c exp(t)
  log_msg  = log((1-a)*exp(t) + a*S) - log(S) - log(1+7a)
which needs only 8 exps + 2 logs per edge (vs 64 exps in the naive form).
No max-subtraction needed: t = log_b[src] - log_msg[rv] has t <= 3.3 and
max_c t >= -5.4, so exp/sum stay well inside f32 range.
"""
import sys
for _p in ('/opt/trn_rl_repo', '/root/.axon_site/_ro/trn_rl_repo'):
    if _p not in sys.path:
        import os
        if os.path.isdir(_p):
            sys.path.insert(0, _p)

import math
import numpy as np

from concourse import bass, mybir
from concourse.masks import make_identity

F32 = mybir.dt.float32
I32 = mybir.dt.int32

NCORES = 8
BLK = 16            # block size for two-level segmented reduction
CH = 16384          # rows per chunk (= 128 partitions x 128 cols)
A = math.exp(-3.0)
LN1MA = math.log(1.0 - A)          # ln(1-a)
SCL_LS = (1.0 + 7.0 * A) / (1.0 - A)  # fold log(1+7a)-log(1-a) into ln(scale*S)
A_OVER = A / (1.0 - A)


def _roundup(x, m):
    return (x + m - 1) // m * m


def preprocess(src, dst, rv, N, C, steps, ch1=16384, ch2=16384):
    """Host-side index preprocessing. Returns (consts dict, per-core input arrays)."""
    E = src.shape[0]
    src = src.astype(np.int64)
    dst = dst.astype(np.int64)
    rv = rv.astype(np.int64)
    NPC = N // NCORES
    LB = _roundup(NPC + 1, 128)          # padded log_b slice rows (128*51 = 6528 for N=50000)
    K51 = LB // 128

    q_of_edge = dst // NPC               # computing/aggregating core per edge
    powner = q_of_edge[rv]               # core that produces M-row for each edge

    # --- sigma order: per core q, edges sorted by (powner, src, rv) ---
    # src-sort within blocks gives the A-gather ascending HBM addresses.
    # B_pq counts
    counts = np.zeros((NCORES, NCORES), dtype=np.int64)  # [q][p]
    for q in range(NCORES):
        mask_q = q_of_edge == q
        pq = powner[mask_q]
        counts[q] = np.bincount(pq, minlength=NCORES)
    B = int(_roundup(counts.max(), 128))

    # slot_of_edge: global edge id -> (its core q known) slot index in [0, 8B)
    order = np.lexsort((rv, src, powner, q_of_edge))  # stable multi-key sort
    # within each (q, p) group the edges are sorted by (src, rv)
    slot_of_edge = np.full(E, -1, dtype=np.int64)
    eq = q_of_edge[order]
    ep = powner[order]
    # rank within (q,p) group:
    grp = eq * NCORES + ep
    # index within group via cumcount
    sort_grp = grp  # already grouped because order sorted by (q, p, ...)
    gstart = np.zeros(NCORES * NCORES + 1, dtype=np.int64)
    np.add.at(gstart, sort_grp + 1, 1)
    gstart = np.cumsum(gstart)
    within = np.arange(E, dtype=np.int64) - gstart[sort_grp]
    slot_of_edge[order] = ep * B + within          # slot within its core's [8B) space
    EP = NCORES * B

    # --- pi order (dst-sorted, degree padded to multiples of BLK) per core ---
    deg = np.bincount(dst, minlength=N).astype(np.int64)
    Ln = ((deg + BLK - 1) // BLK) * BLK
    MAXB = int(max(1, (deg.max() + BLK - 1) // BLK))
    # per-core offsets
    S_q = np.zeros(NCORES, dtype=np.int64)
    O = np.zeros(N, dtype=np.int64)                # offset of node's run within ITS core's pi space
    for q in range(NCORES):
        ln_q = Ln[q * NPC:(q + 1) * NPC]
        off = np.concatenate(([0], np.cumsum(ln_q)[:-1]))
        O[q * NPC:(q + 1) * NPC] = off
        S_q[q] = ln_q.sum()
    SM = int(_roundup(S_q.max(), ch2))

    # pi position of each edge: O[dst] + rank within dst group.
    # rank: use the sigma order (sorted by dst within (q,p) blocks) -> need a
    # global per-dst cumcount; order2 sorted by dst then anything:
    order2 = np.argsort(dst, kind='stable')
    dstart = np.concatenate(([0], np.cumsum(np.bincount(dst, minlength=N))))[:-1]
    within2 = np.arange(E, dtype=np.int64) - dstart[dst[order2]]
    pi_of_edge = np.full(E, -1, dtype=np.int64)
    pi_of_edge[order2] = O[dst[order2]] + within2

    # --- chunk layouts ---
    # P1 chunks: per sigma-block p, sub-chunks of <= CH slots (multiples of 128)
    p1_chunks = []      # (pblk, lo, cs)
    for p in range(NCORES):
        lo = 0
        while lo < B:
            cs = min(ch1, B - lo)
            p1_chunks.append((p, lo, cs))
            lo += cs
    TC1 = EP // 128     # total idx cols for per-slot arrays

    # helper: map a list of per-slot values (sigma slot id s) into the
    # [128, TC1] column layout: chunk-major, within chunk (p, k) p-major.
    def to_cols(vals, nslots, chunks):
        # vals: int array [nslots]; chunks: list of (base_slot, cs) in slot space
        arr = np.zeros((128, nslots // 128), dtype=np.int32)
        co = 0
        for base, cs in chunks:
            kc = cs // 128
            v = vals[base:base + cs].reshape(128, kc)
            arr[:, co:co + kc] = v
            co += kc
        return arr

    p1_slotchunks = [(p * B + lo, cs) for (p, lo, cs) in p1_chunks]

    # P2 chunks: SM rows in chunks of ch2
    p2_chunks = [(r * ch2, ch2) for r in range(SM // ch2)]
    # P3 chunks: per dest q, sub-chunks of B
    p3_chunks = []
    for q in range(NCORES):
        lo = 0
        while lo < B:
            cs = min(ch1, B - lo)
            p3_chunks.append((q, lo, cs))
            lo += cs
    p3_slotchunks = [(q * B + lo, cs) for (q, lo, cs) in p3_chunks]

    NPC_PAD = LB                      # blocksums node rows (incl trash at LB-1)
    TRASH = LB - 1

    BT = B + LB

    KB = ch2 // (128 * BLK)
    consts = dict(N=N, C=C, E=E, NPC=NPC, LB=LB, K51=K51, B=B, EP=EP, SM=SM,
                  MAXB=MAXB, BT=BT, TC1=TC1, steps=steps, CH1=ch1, CH2=ch2, KB=KB,
                  p1_chunks=p1_chunks, p2_chunks=p2_chunks, p3_chunks=p3_chunks)

    # --- per-core arrays ---
    per_core = []
    for q in range(NCORES):
        eids = np.where(q_of_edge == q)[0]
        slots = slot_of_edge[eids]                 # in [0, EP)
        # per-slot arrays (default pads)
        idxA = np.zeros(EP, dtype=np.int64)        # a2a_out row of log_b[src]
        idxProd = np.zeros(EP, dtype=np.int64)     # send-slot -> sigma slot of requested msg (on THIS core)
        r_src = src[eids] // NPC
        idxA[slots] = r_src * BT + B + (src[eids] - r_src * NPC)

        # requests: for each slot f of ANY core with powner(f)==q, the request
        # lands at send-slot (dest_core_of_f, position = slot_of_edge[f] % B)
        # and reads sigma slot of edge rv[f] (which lives on q).
        f_ids = np.where(powner == q)[0]           # edges whose M-row q produces
        dest = q_of_edge[f_ids]
        pos = slot_of_edge[f_ids] - dest * 0       # slot within f's core space
        # f's slot = powner(f)*B + within = q*B + within -> position within block = slot - q*B
        pos_in_block = slot_of_edge[f_ids] - q * B
        sendslot = dest * B + pos_in_block
        idxProd[sendslot] = pi_of_edge[rv[f_ids]]  # pi row of rv[f] on q

        # NOTE: edge rv[f] is aggregated on core q (powner(f)=q), so its pi
        # position indexes q's msig2 (pi-ordered) directly.

        # idxStore: sigma slot -> pi row (msig2 is stored in pi order; pad
        # sigma slots go to the trash row SM; pi pad rows stay zero)
        idxStore = np.full(EP, SM, dtype=np.int64)
        pi_q = pi_of_edge[eids]
        idxStore[slots] = pi_q

        # idxMini: block b -> blocksums row (nloc*MAXB + j) or trash
        nb = SM // BLK
        idxMini = np.full(nb, TRASH * MAXB, dtype=np.int64)
        degq = deg[q * NPC:(q + 1) * NPC]
        Oq = O[q * NPC:(q + 1) * NPC]
        nblocks = (degq + BLK - 1) // BLK
        for_nodes = np.repeat(np.arange(NPC), nblocks)
        jvec = np.concatenate([np.arange(nb_, dtype=np.int64) for nb_ in nblocks]) \
            if len(nblocks) else np.zeros(0, dtype=np.int64)
        bstart = np.repeat(Oq // BLK, nblocks) + jvec
        idxMini[bstart] = for_nodes * MAXB + jvec

        # column layouts
        idxA_c = to_cols(idxA, EP, p1_slotchunks)
        idxStore_c = to_cols(idxStore, EP, p1_slotchunks)
        idxProd_c = to_cols(idxProd, EP, p3_slotchunks)
        # idxMini: [128, (SM/ch2)*KB]: chunk r, partition p, kb -> block id
        nchunk2 = SM // ch2
        kcols = ch2 // 128
        idxMini_c = np.zeros((128, nchunk2 * KB), dtype=np.int32)
        for r in range(nchunk2):
            blk_ids = (r * ch2 + np.arange(ch2).reshape(128, kcols)[:, ::BLK]) // BLK
            idxMini_c[:, r * KB:(r + 1) * KB] = idxMini[blk_ids]

        per_core.append(dict(idxA=idxA_c, idxProd=idxProd_c, idxStore=idxStore_c,
                             idxMini=idxMini_c))
    return consts, per_core


# ---------------------------------------------------------------------------
# Kernel builder
# ---------------------------------------------------------------------------

class P:
    """Op-record program: logical order with resolved semaphore waits."""
    def __init__(self):
        self.records = []          # (eng, waits[(sem,val)], emit_fn, incs[(sem,k)])
        self.cnt = {}

    def n(self, sem):
        return self.cnt.get(sem, 0)

    def op(self, eng, emit, incs=(), waits=()):
        waits = [(s, v) for (s, v) in waits if v > 0]
        self.records.append((eng, waits, emit, list(incs)))
        after = {}
        for s, k in incs:
            self.cnt[s] = self.cnt.get(s, 0) + k
            after[s] = self.cnt[s]
        return after

def build_kernel(consts, repeat=1, ablate=()):
    NPC, LB, K51 = consts['NPC'], consts['LB'], consts['K51']
    B, EP, SM, MAXB, BT, TC1 = (consts['B'], consts['EP'], consts['SM'],
                                consts['MAXB'], consts['BT'], consts['TC1'])
    C = consts['C']
    steps = consts['steps']
    p1_chunks, p2_chunks, p3_chunks = (consts['p1_chunks'], consts['p2_chunks'],
                                       consts['p3_chunks'])
    DIN = consts.get('DIN', 256)
    CH1 = consts.get('CH1', 16384)
    CH2 = consts.get('CH2', 16384)
    KB = consts.get('KB', 8)
    KC2 = CH2 // 128
    NX = LB
    KT = DIN // 128

    nc = bass.Bass(num_devices=NCORES, num_swdge_queues=4)
    qrr = [0]

    def _q(inst):
        # spread indirect DMAs round-robin over the 4 SWDGE queues
        inst.ins.queue = 'qPoolDynamic%s' % ((qrr[0] % 4) or '')
        qrr[0] += 1
        return inst

    x_pad = nc.declare_dram_parameter('x_pad', [NX, DIN], F32, isOutput=False)
    w_in = nc.declare_dram_parameter('w_in', [DIN, C], F32, isOutput=False)
    b_in = nc.declare_dram_parameter('b_in', [128, K51 * C], F32, isOutput=False)
    idxA_in = nc.declare_dram_parameter('idxA', [128, TC1], I32, isOutput=False)
    idxProd_in = nc.declare_dram_parameter('idxProd', [128, TC1], I32, isOutput=False)
    idxStore_in = nc.declare_dram_parameter('idxStore', [128, TC1], I32, isOutput=False)
    idxMini_in = nc.declare_dram_parameter('idxMini', [128, (SM // CH2) * KB], I32, isOutput=False)
    out_t = nc.declare_dram_parameter('out', [LB, C], F32, isOutput=True)

    a2a_in = nc.dram_tensor('a2a_in', [NCORES * BT, C], F32)
    a2a_out = nc.dram_tensor('a2a_out', [NCORES * BT, C], F32)
    msig2 = nc.dram_tensor('msig2', [SM + 128, C], F32)
    bsums = nc.dram_tensor('bsums', [LB * MAXB, C], F32)

    prog = P()
    ACT, DVE, GPS, SYN, PE = 'scalar', 'vector', 'gpsimd', 'sync', 'tensor'

    SEM_NAMES = ['sV', 'sA', 'sP', 'cc',
                 'mA0', 'mA1', 'mB0', 'mB1', 'mD0', 'mD1',
                 'mR0', 'mR1', 'mMini0', 'mMini1', 'mG0', 'mG1', 'mGS0', 'mGS1',
                 'mX0', 'mX1', 'mI', 'mZ', 'mP0', 'mT', 'mO']

    from contextlib import ExitStack
    with ExitStack() as ctx:
        block = ctx.enter_context(nc.Block())
        sems = {n: ctx.enter_context(nc.semaphore(n)) for n in SEM_NAMES}

        def sb(name, shape, dt=F32):
            return ctx.enter_context(nc.sbuf_tensor(name, shape, dt))

        idxA_sb = sb('idxA_sb', [128, TC1], I32)
        idxProd_sb = sb('idxProd_sb', [128, TC1], I32)
        idxStore_sb = sb('idxStore_sb', [128, TC1], I32)
        idxMini_sb = sb('idxMini_sb', [128, (SM // CH2) * KB], I32)
        Ab0 = sb('Ab0', [128, 1024]); Ab1 = sb('Ab1', [128, 1024])
        Bb0 = sb('Bb0', [128, 1024]); Bb1 = sb('Bb1', [128, 1024])
        Cb0 = sb('Cb0', [128, 1024]); Cb1 = sb('Cb1', [128, 1024])
        Db0 = sb('Db0', [128, 1024]); Db1 = sb('Db1', [128, 1024])
        Sb0 = sb('Sb0', [128, 128]); Sb1 = sb('Sb1', [128, 128])
        Lb0 = sb('Lb0', [128, 128]); Lb1 = sb('Lb1', [128, 128])
        Rb0 = sb('Rb0', [128, KC2 * C]); Rb1 = sb('Rb1', [128, KC2 * C])
        BSall = sb('BSall', [128, (SM // CH2) * KB * C])
        Gb0 = sb('Gb0', [128, 1024]); Gb1 = sb('Gb1', [128, 1024])
        BSsb = sb('BSsb', [128, K51 * MAXB * C])
        AGsb = sb('AGsb', [128, K51 * C])
        U2sb = sb('U2sb', [128, K51 * C])
        MXsb = sb('MXsb', [128, K51])
        S3sb = sb('S3sb', [128, K51])
        logb0_sb = sb('logb0_sb', [128, K51 * C])
        logb_sb = sb('logb_sb', [128, K51 * C])
        xc0 = sb('xc0', [128, 256]); xc1 = sb('xc1', [128, 256])
        xT_sb = sb('xT_sb', [128, 128])
        W_sb = sb('W_sb', [128, KT * C])
        bB_sb = sb('bB_sb', [128, K51 * C])
        ident = sb('ident', [128, 128])
        mlog8_sb = sb('mlog8_sb', [128, 1024])
        zero_sb = sb('zero_sb', [128, 1024])
        biasE = sb('biasE', [128, 1])
        psT = ctx.enter_context(nc.psum_tensor([128, 128], F32))
        psL = ctx.enter_context(nc.psum_tensor([128, C], F32))

        AB = [Ab0, Ab1]; BB = [Bb0, Bb1]; CB = [Cb0, Cb1]; DB = [Db0, Db1]
        SB = [Sb0, Sb1]; LB_ = [Lb0, Lb1]; RB = [Rb0, Rb1]
        GB = [Gb0, Gb1]; XCH = [xc0, xc1]

        pr = prog

        # ================= INIT =================
        pr.op(SYN, lambda E: E.dma_start(out=idxA_sb[:], in_=idxA_in[:]), [('mI', 16)])
        pr.op(SYN, lambda E: E.dma_start(out=idxProd_sb[:], in_=idxProd_in[:]), [('mI', 16)])
        pr.op(SYN, lambda E: E.dma_start(out=idxStore_sb[:], in_=idxStore_in[:]), [('mI', 16)])
        pr.op(SYN, lambda E: E.dma_start(out=idxMini_sb[:], in_=idxMini_in[:]), [('mI', 16)])
        pr.op(SYN, lambda E: E.dma_start(
            out=W_sb[:].rearrange('p (t c) -> p t c', t=KT),
            in_=w_in[:].rearrange('(t p) c -> p t c', p=128)), [('mI', 16)])
        pr.op(SYN, lambda E: E.dma_start(out=bB_sb[:], in_=b_in[:]), [('mI', 16)])
        mI_tot = pr.n('mI')

        pr.op(GPS, lambda E: E.memset(mlog8_sb[:], -math.log(C)), [('sP', 1)])
        pr.op(GPS, lambda E: E.memset(zero_sb[:], 0.0), [('sP', 1)])
        pr.op(GPS, lambda E: E.memset(biasE[:], LN1MA), [('sP', 1)])
        pr.op(GPS, lambda E: E.memset(ident[:], 0.0), [('sP', 1)])
        pr.op(GPS, lambda E: E.affine_select(
            out=ident[:], in_=ident[:],
            compare_op=mybir.AluOpType.not_equal, fill=1.0, base=0,
            pattern=[[-1, 128]], channel_multiplier=1), [('sP', 1)])
        const_sp = pr.n('sP')

        # zero msig2 (pi pads + trash row) + bsums table (once per launch)
        lo = 0
        nz2 = SM + 128
        while lo < nz2:
            csz = min(CH, nz2 - lo) // 128 * 128
            if csz == 0:
                break
            pr.op(SYN, lambda E, lo=lo, csz=csz: E.dma_start(
                out=msig2[lo:lo + csz, :].rearrange('(p k) d -> p (k d)', p=128),
                in_=zero_sb[:, :csz // 128 * C]),
                [('mZ', 16)], waits=[('sP', 2)])
            lo += csz
        nz = LB * MAXB
        lo = 0
        while lo < nz:
            csz = min(CH, nz - lo)
            csz = csz // 128 * 128
            if csz == 0:
                break
            pr.op(SYN, lambda E, lo=lo, csz=csz: E.dma_start(
                out=bsums[lo:lo + csz, :].rearrange('(p k) d -> p (k d)', p=128),
                in_=zero_sb[:, :csz // 128 * C]),
                [('mZ', 16)], waits=[('sP', 2)])
            lo += csz
        mZ_tot = pr.n('mZ')

        # logb0 = logsoftmax(x @ W + b)
        for t in range(K51):
            j = t % 2
            wX = [('mX%d' % j, 0)]
            if t >= 2:
                wX = [('sP', pr.n('sP'))]      # all PE work through t-1 done
            pr.op(SYN, lambda E, t=t, j=j: E.dma_start(
                out=XCH[j][:], in_=x_pad[t * 128:(t + 1) * 128, :]),
                [('mX%d' % j, 16)], waits=[(s, v) for (s, v) in wX if v > 0])
            xthr = pr.n('mX%d' % j)
            for h in range(KT):
                wt = [('mX%d' % j, xthr), ('sV', pr.n('sV'))]
                if t == 0 and h == 0:
                    wt.append(('sP', const_sp))
                pr.op(PE, lambda E, j=j, h=h: E.transpose(
                    out=psT[:], in_=XCH[j][:, h * 128:(h + 1) * 128], identity=ident[:]),
                    [('sP', 1)], waits=wt)
                pr.op(DVE, lambda E: E.tensor_copy(out=xT_sb[:], in_=psT[:]),
                      [('sV', 1)], waits=[('sP', pr.n('sP'))])
                pr.op(PE, lambda E, h=h: E.matmul(
                    out=psL[:], lhsT=xT_sb[:], rhs=W_sb[:, h * C:(h + 1) * C],
                    start=(h == 0), stop=(h == KT - 1)),
                    [('sP', 1)], waits=[('sV', pr.n('sV'))])
            pr.op(DVE, lambda E, t=t: E.tensor_copy(
                out=logb0_sb[:, t * C:(t + 1) * C], in_=psL[:]),
                [('sV', 1)], waits=[('sP', pr.n('sP'))])
        pr.op(DVE, lambda E: E.tensor_add(out=logb0_sb[:], in0=logb0_sb[:], in1=bB_sb[:]),
              [('sV', 1)], waits=[('mI', mI_tot)])
        pr.op(ACT, lambda E: E.activation(
            out=U2sb[:], in_=logb0_sb[:], func=mybir.ActivationFunctionType.Exp),
            [('sA', 1)], waits=[('sV', pr.n('sV'))])
        pr.op(DVE, lambda E: E.tensor_reduce(
            out=S3sb[:], in_=U2sb[:].rearrange('p (k c) -> p k c', c=C),
            axis=mybir.AxisListType.X, op=mybir.AluOpType.add),
            [('sV', 1)], waits=[('sA', pr.n('sA'))])
        pr.op(ACT, lambda E: E.activation(
            out=S3sb[:], in_=S3sb[:], func=mybir.ActivationFunctionType.Ln),
            [('sA', 1)], waits=[('sV', pr.n('sV'))])
        pr.op(DVE, lambda E: E.tensor_sub(
            out=logb0_sb[:].rearrange('p (k c) -> p k c', c=C),
            in0=logb0_sb[:].rearrange('p (k c) -> p k c', c=C),
            in1=S3sb[:].to_broadcast([128, K51, C])),
            [('sV', 1)], waits=[('sA', pr.n('sA'))])
        logb0_sv = pr.n('sV')

        # ================= REPEAT =================
        for rep in range(repeat):
            # P0: fill a2a_in M-regions with -log C; logb0 slices into tails
            gA_tots = [pr.n('mA0'), pr.n('mA1'), pr.n('mB0'), pr.n('mB1')]
            for q in range(NCORES):
                lo = 0
                while lo < B:
                    csz = min(CH, B - lo)
                    pr.op(SYN, lambda E, q=q, lo=lo, csz=csz: E.dma_start(
                        out=a2a_in[q * BT + lo:q * BT + lo + csz, :]
                            .rearrange('(p k) d -> p (k d)', p=128),
                        in_=mlog8_sb[:, :csz // 128 * C]),
                        [('mP0', 16)],
                        waits=[('sP', 2),
                               ('mA0', gA_tots[0]), ('mA1', gA_tots[1]),
                               ('mB0', gA_tots[2]), ('mB1', gA_tots[3])])
                    lo += csz
                pr.op(SYN, lambda E, q=q: E.dma_start(
                    out=a2a_in[q * BT + B:q * BT + B + LB, :]
                        .rearrange('(k p) d -> p k d', p=128),
                    in_=logb0_sb[:].rearrange('p (k d) -> p k d', d=C)),
                    [('mP0', 16)], waits=[('sV', logb0_sv)])
            pr.op(GPS, lambda E: E.collective_compute(
                'AllToAll', mybir.AluOpType.bypass,
                replica_groups=[list(range(NCORES))],
                ins=[a2a_in[:]], outs=[a2a_out[:]]),
                [('cc', 1)],
                waits=[('mP0', pr.n('mP0')), ('mZ', mZ_tot), ('mI', mI_tot),
                       ('mGS0', pr.n('mGS0')), ('mGS1', pr.n('mGS1')),
                       ('mO', pr.n('mO'))])
            cc_ready = pr.n('cc')

            for step in range(1, steps + 1):
                prev_mini = [('mMini0', pr.n('mMini0')),
                             ('mMini1', pr.n('mMini1'))]
                # ---- P1 ----
                subdone = {}; lndone = {}; stordone = {}
                co = 0
                pend = None         # deferred pi-scatter of the previous chunk

                def flush_scatter():
                    # emit the deferred chunk's per-column scatters to msig2
                    nonlocal pend
                    if pend is None:
                        return
                    (pci, pj, pco, pkc, pdready) = pend
                    sDp = 'mD%d' % pj
                    if 'store' not in ablate:
                        for kk in range(pkc):
                            pr.op(GPS, lambda E, pj=pj, pco=pco, kk=kk:
                                  _q(E.indirect_dma_start(
                                      out=msig2[:],
                                      out_offset=bass.IndirectOffsetOnAxis(
                                          ap=idxStore_sb[:, pco + kk:pco + kk + 1],
                                          axis=0),
                                      in_=DB[pj][:, kk * C:(kk + 1) * C],
                                      in_offset=None)),
                                  [(sDp, 16)],
                                  waits=([('sV', pdready)] if kk == 0 else []))
                    stordone[pci] = pr.n(sDp)
                    pend = None

                for ci, (pblk, lo, cs) in enumerate(p1_chunks):
                    j = ci % 2
                    kc = cs // 128
                    sA_ = 'mA%d' % j; sB_ = 'mB%d' % j; sD_ = 'mD%d' % j
                    wA = [('cc', cc_ready)]
                    wB = [('cc', cc_ready)]
                    if ci >= 2:
                        wA.append(('sV', subdone[ci - 2]))
                        wB.append(('sV', subdone[ci - 2]))
                    if 'a' not in ablate:
                        for kk in range(kc):
                            pr.op(GPS, lambda E, j=j, co=co, kk=kk: _q(E.indirect_dma_start(
                                out=AB[j][:, kk * C:(kk + 1) * C], out_offset=None,
                                in_=a2a_out[:],
                                in_offset=bass.IndirectOffsetOnAxis(
                                    ap=idxA_sb[:, co + kk:co + kk + 1], axis=0))),
                                [(sA_, 16)], waits=(wA if kk == 0 else []))
                    thrA = pr.n(sA_)
                    flush_scatter()
                    pr.op(SYN, lambda E, j=j, pblk=pblk, lo=lo, cs=cs, kc=kc: E.dma_start(
                        out=BB[j][:, :kc * C],
                        in_=a2a_out[pblk * BT + lo:pblk * BT + lo + cs, :]
                            .rearrange('(p k) d -> p (k d)', p=128)),
                        [(sB_, 16)], waits=wB)
                    thrB = pr.n(sB_)
                    wC = [(sA_, thrA), (sB_, thrB)]
                    if ci >= 2:
                        wC.append(('sA', lndone[ci - 2]))
                    pr.op(DVE, lambda E, j=j, kc=kc: E.tensor_sub(
                        out=CB[j][:, :kc * C], in0=AB[j][:, :kc * C],
                        in1=BB[j][:, :kc * C]),
                        [('sV', 1)], waits=wC)
                    subdone[ci] = pr.n('sV')
                    we = [('sV', subdone[ci])]
                    if rep == 0 and step == 1 and ci == 0:
                        we.append(('sP', const_sp))
                    pr.op(ACT, lambda E, j=j, kc=kc: E.activation(
                        out=CB[j][:, :kc * C], in_=CB[j][:, :kc * C],
                        func=mybir.ActivationFunctionType.Exp, bias=biasE[:, :1]),
                        [('sA', 1)], waits=we)
                    pr.op(DVE, lambda E, j=j, kc=kc: E.tensor_reduce(
                        out=SB[j][:, :kc],
                        in_=CB[j][:, :kc * C].rearrange('p (k c) -> p k c', c=C),
                        axis=mybir.AxisListType.X, op=mybir.AluOpType.add),
                        [('sV', 1)], waits=[('sA', pr.n('sA'))])
                    pr.op(ACT, lambda E, j=j, kc=kc: E.activation(
                        out=LB_[j][:, :kc], in_=SB[j][:, :kc],
                        func=mybir.ActivationFunctionType.Ln, scale=SCL_LS),
                        [('sA', 1)], waits=[('sV', pr.n('sV'))])
                    pr.op(DVE, lambda E, j=j, kc=kc: E.tensor_scalar_mul(
                        out=SB[j][:, :kc], in0=SB[j][:, :kc], scalar1=A_OVER),
                        [('sV', 1)], waits=[('sA', pr.n('sA'))])
                    pr.op(DVE, lambda E, j=j, kc=kc: E.tensor_add(
                        out=CB[j][:, :kc * C].rearrange('p (k c) -> p k c', c=C),
                        in0=CB[j][:, :kc * C].rearrange('p (k c) -> p k c', c=C),
                        in1=SB[j][:, :kc].to_broadcast([128, kc, C])),
                        [('sV', 1)])
                    pr.op(ACT, lambda E, j=j, kc=kc: E.activation(
                        out=CB[j][:, :kc * C], in_=CB[j][:, :kc * C],
                        func=mybir.ActivationFunctionType.Ln),
                        [('sA', 1)], waits=[('sV', pr.n('sV'))])
                    lndone[ci] = pr.n('sA')
                    wD = [('sA', lndone[ci])]
                    if ci >= 2:
                        wD.append((sD_, stordone[ci - 2]))
                    pr.op(DVE, lambda E, j=j, kc=kc: E.tensor_sub(
                        out=DB[j][:, :kc * C].rearrange('p (k c) -> p k c', c=C),
                        in0=CB[j][:, :kc * C].rearrange('p (k c) -> p k c', c=C),
                        in1=LB_[j][:, :kc].to_broadcast([128, kc, C])),
                        [('sV', 1)], waits=wD)
                    dready = pr.n('sV')
                    pend = (ci, j, co, kc, dready)
                    co += kc
                flush_scatter()
                p1_store_tots = [('mD0', pr.n('mD0')), ('mD1', pr.n('mD1'))]
                p1_gather_tots = [('mA0', pr.n('mA0')), ('mA1', pr.n('mA1')),
                                  ('mB0', pr.n('mB0')), ('mB1', pr.n('mB1'))]

                # ---- P2a: loads + blocksum reduces (Minis deferred past P3
                # so their waits don't stall G-gather descriptor generation) ----
                bsdone = {}
                for r, (base2, cs2) in enumerate(p2_chunks):
                    j = r % 2
                    sR_ = 'mR%d' % j
                    w = list(p1_store_tots)
                    if r >= 2:
                        w.append(('sV', bsdone[r - 2]))
                    pr.op(SYN, lambda E, j=j, base2=base2, cs2=cs2: E.dma_start(
                        out=RB[j][:],
                        in_=msig2[base2:base2 + cs2, :]
                            .rearrange('(p k) d -> p (k d)', p=128)),
                        [(sR_, 16)], waits=w)
                    thrR = pr.n(sR_)
                    pr.op(DVE, lambda E, j=j, r=r: E.tensor_reduce(
                        out=BSall[:, r * KB * C:(r + 1) * KB * C]
                            .rearrange('p (kb c) -> p kb c', c=C),
                        in_=RB[j][:].rearrange('p (kb t c) -> p kb c t', kb=KB, t=BLK, c=C),
                        axis=mybir.AxisListType.X, op=mybir.AluOpType.add),
                        [('sV', 1)],
                        waits=[(sR_, thrR)] + list(prev_mini))
                    bsdone[r] = pr.n('sV')

                # ---- P3 (not last step) ----
                if step < steps:
                    co3 = 0
                    g3done = {}
                    for ci, (q, lo, cs) in enumerate(p3_chunks):
                        j = ci % 2
                        sG_ = 'mG%d' % j; sGS_ = 'mGS%d' % j
                        kc = cs // 128
                        w = list(p1_store_tots)
                        if ci >= 2:
                            w.append((sGS_, g3done[ci - 2]))
                        if 'p3' not in ablate:
                            for kk in range(kc):
                                pr.op(GPS, lambda E, j=j, co3=co3, kk=kk: _q(E.indirect_dma_start(
                                    out=GB[j][:, kk * C:(kk + 1) * C], out_offset=None,
                                    in_=msig2[:],
                                    in_offset=bass.IndirectOffsetOnAxis(
                                        ap=idxProd_sb[:, co3 + kk:co3 + kk + 1], axis=0))),
                                    [(sG_, 16)], waits=(w if kk == 0 else []))
                        thrG = pr.n(sG_)
                        pr.op(SYN, lambda E, j=j, q=q, lo=lo, cs=cs, kc=kc: E.dma_start(
                            out=a2a_in[q * BT + lo:q * BT + lo + cs, :]
                                .rearrange('(p k) d -> p (k d)', p=128),
                            in_=GB[j][:, :kc * C]),
                            [(sGS_, 16)], waits=[(sG_, thrG)])
                        g3done[ci] = pr.n(sGS_)
                        co3 += kc

                # ---- P2b: blocksum scatters (after P3 so G desc-gen flows) ----
                for r in range(len(p2_chunks)):
                    j = r % 2
                    if 'mini' in ablate:
                        continue
                    for kb in range(KB):
                        pr.op(GPS, lambda E, r=r, kb=kb: _q(E.indirect_dma_start(
                            out=bsums[:],
                            out_offset=bass.IndirectOffsetOnAxis(
                                ap=idxMini_sb[:, r * KB + kb:r * KB + kb + 1], axis=0),
                            in_=BSall[:, (r * KB + kb) * C:(r * KB + kb + 1) * C],
                            in_offset=None)),
                            [('mMini%d' % j, 16)],
                            waits=([('sV', bsdone[r])] if kb == 0 else []))
                mini_tots = [('mMini0', pr.n('mMini0')), ('mMini1', pr.n('mMini1'))]

                # ---- P4 ----
                pr.op(SYN, lambda E: E.dma_start(
                    out=BSsb[:].rearrange('p (k j d) -> p k j d', j=MAXB, d=C),
                    in_=bsums[:].rearrange('(k p j) d -> p k j d', p=128, j=MAXB)),
                    [('mT', 16)], waits=mini_tots)
                pr.op(DVE, lambda E: E.tensor_reduce(
                    out=AGsb[:].rearrange('p (k c) -> p k c', c=C),
                    in_=BSsb[:].rearrange('p (k j c) -> p k c j', k=K51, j=MAXB, c=C),
                    axis=mybir.AxisListType.X, op=mybir.AluOpType.add),
                    [('sV', 1)], waits=[('mT', pr.n('mT'))])
                pr.op(DVE, lambda E: E.tensor_add(
                    out=AGsb[:], in0=AGsb[:], in1=logb0_sb[:]),
                    [('sV', 1)])
                pr.op(DVE, lambda E: E.tensor_reduce(
                    out=MXsb[:], in_=AGsb[:].rearrange('p (k c) -> p k c', c=C),
                    axis=mybir.AxisListType.X, op=mybir.AluOpType.max),
                    [('sV', 1)])
                pr.op(DVE, lambda E: E.tensor_sub(
                    out=U2sb[:].rearrange('p (k c) -> p k c', c=C),
                    in0=AGsb[:].rearrange('p (k c) -> p k c', c=C),
                    in1=MXsb[:].to_broadcast([128, K51, C])),
                    [('sV', 1)])
                pr.op(ACT, lambda E: E.activation(
                    out=U2sb[:], in_=U2sb[:], func=mybir.ActivationFunctionType.Exp),
                    [('sA', 1)], waits=[('sV', pr.n('sV'))])
                pr.op(DVE, lambda E: E.tensor_reduce(
                    out=S3sb[:], in_=U2sb[:].rearrange('p (k c) -> p k c', c=C),
                    axis=mybir.AxisListType.X, op=mybir.AluOpType.add),
                    [('sV', 1)], waits=[('sA', pr.n('sA'))])
                pr.op(ACT, lambda E: E.activation(
                    out=S3sb[:], in_=S3sb[:], func=mybir.ActivationFunctionType.Ln),
                    [('sA', 1)], waits=[('sV', pr.n('sV'))])
                pr.op(DVE, lambda E: E.tensor_add(
                    out=S3sb[:], in0=S3sb[:], in1=MXsb[:]),
                    [('sV', 1)], waits=[('sA', pr.n('sA'))])
                pr.op(DVE, lambda E: E.tensor_sub(
                    out=logb_sb[:].rearrange('p (k c) -> p k c', c=C),
                    in0=AGsb[:].rearrange('p (k c) -> p k c', c=C),
                    in1=S3sb[:].to_broadcast([128, K51, C])),
                    [('sV', 1)])
                logb_sv = pr.n('sV')

                if step < steps:
                    for q in range(NCORES):
                        pr.op(SYN, lambda E, q=q: E.dma_start(
                            out=a2a_in[q * BT + B:q * BT + B + LB, :]
                                .rearrange('(k p) d -> p k d', p=128),
                            in_=logb_sb[:].rearrange('p (k d) -> p k d', d=C)),
                            [('mO', 16)], waits=[('sV', logb_sv)])
                    a2a_waits = [('mO', pr.n('mO')),
                                 ('mGS0', pr.n('mGS0')), ('mGS1', pr.n('mGS1'))]
                    a2a_waits += p1_gather_tots
                    pr.op(GPS, lambda E: E.collective_compute(
                        'AllToAll', mybir.AluOpType.bypass,
                        replica_groups=[list(range(NCORES))],
                        ins=[a2a_in[:]], outs=[a2a_out[:]]),
                        [('cc', 1)], waits=a2a_waits)
                    cc_ready = pr.n('cc')
                else:
                    pr.op(SYN, lambda E: E.dma_start(
                        out=out_t[:].rearrange('(k p) d -> p k d', p=128),
                        in_=logb_sb[:].rearrange('p (k d) -> p k d', d=C)),
                        [('mO', 16)], waits=[('sV', logb_sv)])

        # final totals for drains
        finals = {n: prog.n(n) for n in SEM_NAMES}

        serial_sem = {DVE: 'sV', ACT: 'sA', PE: 'sP', GPS: 'sP'}
        SYN_SEMS = ['mI', 'mZ', 'mP0', 'mB0', 'mB1', 'mR0', 'mR1',
                    'mGS0', 'mGS1', 'mX0', 'mX1', 'mT', 'mO']
        GPS_SEMS = ['mA0', 'mA1', 'mD0', 'mD1', 'mMini0', 'mMini1', 'mG0', 'mG1', 'cc']

        def emit_for(eng_name):
            def fn(E):
                cnt = {}
                last_serial = 0
                ssem = serial_sem.get(eng_name)
                for (eng, waits, emit, incs) in prog.records:
                    if eng != eng_name:
                        for s, k in incs:
                            cnt[s] = cnt.get(s, 0) + k
                        continue
                    wl = list(waits)
                    if ssem is not None and any(s == ssem for s, _ in incs):
                        if last_serial > 0:
                            wl.append((ssem, last_serial))
                    for s, v in wl:
                        E.wait_ge(sems[s], v)
                    inst = emit(E)
                    for s, k in incs:
                        inst.then_inc(sems[s], k)
                        cnt[s] = cnt.get(s, 0) + k
                        if s == ssem:
                            last_serial = cnt[s]
                if eng_name == SYN:
                    for s in SYN_SEMS:
                        if finals[s] > 0:
                            E.wait_ge(sems[s], finals[s])
                if eng_name == GPS:
                    for s in GPS_SEMS:
                        if finals[s] > 0:
                            E.wait_ge(sems[s], finals[s])
            return fn

        block.sync(emit_for(SYN))
        block.vector(emit_for(DVE))
        block.scalar(emit_for(ACT))
        block.gpsimd(emit_for(GPS))
        block.tensor(emit_for(PE))

    return nc

# ---------------------------------------------------------------------------
# Host entry
# ---------------------------------------------------------------------------

_CACHE = {}


def _build_inputs(x, W, b, consts, per_core):
    N, NPC, LB, K51 = consts['N'], consts['NPC'], consts['LB'], consts['K51']
    C = consts['C']
    in_maps = []
    for q in range(NCORES):
        xs = np.zeros((LB, x.shape[1]), np.float32)
        xs[:NPC] = x[q * NPC:(q + 1) * NPC]
        b_b = np.tile(b.astype(np.float32)[None, :], (128, K51))
        m = dict(x_pad=xs, w_in=W.astype(np.float32), b_in=b_b,
                 idxA=per_core[q]['idxA'], idxProd=per_core[q]['idxProd'],
                 idxStore=per_core[q]['idxStore'], idxMini=per_core[q]['idxMini'])
        in_maps.append(m)
    return in_maps


def kernel(x, W, b, edge_index, rv, repeat=1, use_sim=False, steps=5,
           ch1=16384, ch2=16384):
    x = np.asarray(x); W = np.asarray(W); b = np.asarray(b)
    edge_index = np.asarray(edge_index); rv = np.asarray(rv)
    N, C = x.shape[0], W.shape[1]
    src, dst = edge_index[0], edge_index[1]

    key = (N, C, src.shape[0], steps, repeat, ch1, ch2,
           hash(src.tobytes()) ^ hash(dst.tobytes()) ^ hash(rv.tobytes()))
    if key in _CACHE:
        consts, per_core, run = _CACHE[key]
    else:
        consts, per_core = preprocess(src, dst, rv, N, C, steps, ch1=ch1, ch2=ch2)
        consts['DIN'] = x.shape[1]
        nc = build_kernel(consts, repeat=repeat)
        if use_sim:
            run = ('sim', nc)
        else:
            from concourse.bass_utils import run_bass_kernel_spmd
            run = ('hw', nc)
        _CACHE[key] = (consts, per_core, run)

    in_maps = _build_inputs(x, W, b, consts, per_core)
    mode, nc = run
    NPC = consts['N'] // NCORES
    if mode == 'sim':
        from concourse.bass_interp import MultiCoreSim
        sim = MultiCoreSim(nc, num_cores=NCORES)
        for q in range(NCORES):
            for k, v in in_maps[q].items():
                sim.cores[q].tensor(k)[:] = v
        sim.simulate()
        outs = [np.array(sim.cores[q].tensor('out')) for q in range(NCORES)]
    else:
        from concourse.bass_utils import run_bass_kernel_spmd
        res = run_bass_kernel_spmd(nc, in_maps, list(range(NCORES)))
        outs = [res.results[q]['out'] for q in range(NCORES)]
    return np.concatenate([o[:NPC] for o in outs], axis=0)

